# revision 1
# baseline (speedup 1.0000x reference)
"""LocalPoolPointnet on 8 Trainium2 cores.

Sharding: points are sorted by their (batch, sparse-bin) key on the host and
split so every core owns a contiguous bin range of one batch plus all points
that fall into it.  The scatter_mean / gather pairs then become fully
core-local one-hot matmuls (no collectives).  All floating-point model math
(coordinate fractional offsets, MLPs, gelu, segment means) runs on device;
the host only computes integer routing metadata (bin ids, permutations) and
re-assembles the output.
"""

import sys
import numpy as np

# ---------------------------------------------------------------- constants
B = 2
NP_ = 100_000
HID = 128
D2 = 256
NBLK = 5
RES = 64
R = 20_005          # max_coord_num in the reference
BIG = RES ** 3 + 1
NCORES = 8
CORES_PER_BATCH = NCORES // B

NPTS = 25_600       # padded points per core  (= 50 * 512 = 200 * 128)
NCHUNK = NPTS // 512            # 49
NTILES = NPTS // 128            # 196
WIN = 512                       # scatter/gather bin window per 512-pt chunk
NBINS = 6_016                   # padded bins per core (= 47 * 128)
NBIAS = 13                      # b_pos(2) b0(5) b1(5) b_c(1)
FW = NPTS * 4 // 128            # 784

F32 = np.float32


# ================================================================ host prep
def point_meta(p, sparse_coords, res):
    """Integer routing metadata, bit-identical to the reference's indexing."""
    p = np.asarray(p, F32)
    sc = np.asarray(sparse_coords)
    coord = np.clip(p + F32(0.5), F32(1e-6), F32(1.0 - 1e-6)) * F32(res)
    cl = coord.astype(np.int32)
    lin = (cl[..., 0] * res + cl[..., 1]) * res + cl[..., 2]      # [B, NP]

    slin = (sc[:, 1] * res + sc[:, 2]) * res + sc[:, 3]
    index = np.empty((B, NP_), np.int64)
    for b in range(B):
        coords_b = np.sort(np.where(sc[:, 0] == b, slin, BIG))
        index[b] = np.searchsorted(coords_b, lin[b], side="left")
    counts = np.bincount(sc[:, 0], minlength=B)
    return index, counts


def shard(p, index):
    """Split each batch's points into CORES_PER_BATCH contiguous-bin shards."""
    shards = []
    for b in range(B):
        idx = index[b]
        order = np.argsort(idx, kind="stable")
        sidx = idx[order]
        binc = np.bincount(idx, minlength=R)
        csum = np.cumsum(binc)
        prev_hi = 0
        for c in range(CORES_PER_BATCH):
            if c < CORES_PER_BATCH - 1:
                target = (c + 1) * NP_ // CORES_PER_BATCH
                hi = int(np.searchsorted(csum, target))
                # csum[hi-1] < target <= csum[hi]; pick the closer boundary
                if hi > 0 and target - csum[hi - 1] < csum[hi] - target:
                    hi -= 1
                hi += 1          # shard owns bins [lo, hi)
            else:
                hi = R
            lo = prev_hi
            prev_hi = hi
            sel = slice(int(np.searchsorted(sidx, lo)), int(np.searchsorted(sidx, hi)))
            pts = p[b][order[sel]]                     # [n, 3] sorted by bin
            rel = (sidx[sel] - lo).astype(np.int64)    # sorted rel bins
            assert pts.shape[0] <= NPTS, f"core shard too big: {pts.shape[0]}"
            nb = hi - lo
            assert nb <= NBINS - WIN, f"bin shard too big: {nb}"
            shards.append(dict(batch=b, lo=lo, hi=hi, pts=pts, rel=rel, nb=nb))
    return shards


def core_inputs(sh):
    """Per-core padded arrays for the device kernel."""
    n = sh["pts"].shape[0]
    pts = np.full((NPTS, 3), 0.25, F32)
    pts[:n] = sh["pts"]
    rel = sh["rel"]

    lb = np.full(NPTS, -1.0, F32)       # bin - window base (-1 for dummies)
    wbase = np.zeros(NCHUNK, np.int32)  # window base per 512-pt chunk
    for c in range(NCHUNK):
        s, e = c * 512, min((c + 1) * 512, n)
        if s >= n:
            break
        base = int(rel[s])
        span = int(rel[e - 1]) - base + 1
        assert span <= WIN, f"window overflow: span={span}"
        wbase[c] = base
        lb[s:e] = (rel[s:e] - base).astype(F32)

    cnt = np.bincount(rel, minlength=NBINS).astype(F32)
    recip = F32(1.0) / np.maximum(cnt, F32(1.0))
    rp = np.zeros(NPTS, F32)            # per-point 1/count (0 for dummies)
    rp[:n] = recip[rel]

    # layouts the device wants
    pts4 = np.zeros((4, NPTS), F32)
    pts4[:3] = pts.T
    pts_flat = np.ascontiguousarray(pts4).reshape(128, FW)
    lbT = np.ascontiguousarray(lb.reshape(NTILES, 128).T)          # [128, NTILES]
    lb_rows = np.zeros((64, 512), F32)
    lb_rows[:NCHUNK] = lb.reshape(NCHUNK, 512)
    wb = np.zeros((1, 64), np.int32)
    wb[0, :NCHUNK] = wbase
    rpT = np.ascontiguousarray(rp.reshape(NTILES, 128).T)
    rp_rows = np.zeros((64, 512), F32)
    rp_rows[:NCHUNK] = rp.reshape(NCHUNK, 512)
    return dict(pts_flat=pts_flat, lbT=lbT, lb_rows=lb_rows, wbase=wb,
                rpT=rpT, rp_rows=rp_rows)


def weight_inputs(W_pos, b_pos, W0, b0, W1, b1, Ws, Wc, b_c):
    W_pos, W0, W1, Ws, Wc = [np.ascontiguousarray(x, F32)
                             for x in (W_pos, W0, W1, Ws, Wc)]
    wpos4 = np.zeros((4, D2), F32)
    wpos4[:3] = W_pos
    bias = np.zeros((128, NBIAS), F32)
    bias[:, 0] = np.asarray(b_pos, F32)[:128]
    bias[:, 1] = np.asarray(b_pos, F32)[128:]
    bias[:, 2:7] = np.asarray(b0, F32).T
    bias[:, 7:12] = np.asarray(b1, F32).T
    bias[:, 12] = np.asarray(b_c, F32)
    iota4 = np.zeros((128, 4), F32)
    for j in range(4):
        iota4[:, j] = np.arange(128) + 128 * j
    iota_bc = np.broadcast_to(np.arange(WIN, dtype=F32), (128, WIN)).copy()
    ident = np.eye(128, dtype=F32)
    return dict(wpos4=wpos4, w0=W0, w1=W1, ws=Ws, wc=Wc, bias=bias,
                iota4=iota4, iota_bc=iota_bc, ident=ident)


# ================================================================ bass build
def build_bass():
    if "/opt/trn_rl_repo" not in sys.path:
        sys.path.insert(0, "/opt/trn_rl_repo")
    import concourse.bass as bass
    import concourse.mybir as mybir
    from concourse import bacc, tile, library_config
    from contextlib import ExitStack

    dt = mybir.dt.float32
    AF = mybir.ActivationFunctionType
    OP = mybir.AluOpType
    GELU = AF.Gelu_apprx_tanh
    EV = mybir.EngineType

    nc = bacc.Bacc("TRN2")
    # -------- dram io
    d_pts = nc.dram_tensor("pts_flat", [128, FW], dt, kind="ExternalInput")
    d_lbT = nc.dram_tensor("lbT", [128, NTILES], dt, kind="ExternalInput")
    d_lbr = nc.dram_tensor("lb_rows", [64, 512], dt, kind="ExternalInput")
    d_wb = nc.dram_tensor("wbase", [1, 64], mybir.dt.int32, kind="ExternalInput")
    d_rpT = nc.dram_tensor("rpT", [128, NTILES], dt, kind="ExternalInput")
    d_rpr = nc.dram_tensor("rp_rows", [64, 512], dt, kind="ExternalInput")
    d_wpos4 = nc.dram_tensor("wpos4", [4, D2], dt, kind="ExternalInput")
    d_w0 = nc.dram_tensor("w0", [NBLK, D2, HID], dt, kind="ExternalInput")
    d_w1 = nc.dram_tensor("w1", [NBLK, HID, HID], dt, kind="ExternalInput")
    d_ws = nc.dram_tensor("ws", [NBLK, D2, HID], dt, kind="ExternalInput")
    d_wc = nc.dram_tensor("wc", [HID, HID], dt, kind="ExternalInput")
    d_bias = nc.dram_tensor("bias", [128, NBIAS], dt, kind="ExternalInput")
    d_iota4 = nc.dram_tensor("iota4", [128, 4], dt, kind="ExternalInput")
    d_iotab = nc.dram_tensor("iota_bc", [128, WIN], dt, kind="ExternalInput")
    d_ident = nc.dram_tensor("ident", [128, 128], dt, kind="ExternalInput")
    d_out = nc.dram_tensor("out_grid", [128, NBINS], dt, kind="ExternalOutput")
    d_scr = nc.dram_tensor("pt_scratch", [4, NPTS], dt)   # internal scratch

    with tile.TileContext(nc) as tc, ExitStack() as ctx:
        cpool = ctx.enter_context(tc.tile_pool(name="const", bufs=1))
        spool = ctx.enter_context(tc.tile_pool(name="stage", bufs=2))
        psum2 = ctx.enter_context(tc.tile_pool(name="psum2", bufs=2, space="PSUM"))
        psum1 = ctx.enter_context(tc.tile_pool(name="psum1", bufs=1, space="PSUM"))

        # one reusable window-base register per consumer engine
        breg = {ev: nc.alloc_registers(f"wbase_{ev.name}", engines=(ev,))
                for ev in (EV.DVE, EV.Activation)}

        def load_base(c, ev):
            nc.engines[ev].reg_load(breg[ev], wb[0:1, c:c + 1])
            return nc.snap(breg[ev], donate=True, min_val=0,
                           max_val=NBINS - WIN)

        # ---------------- persistent sbuf
        net = cpool.tile([128, NPTS], dt, tag="net")
        sums = cpool.tile([128, NBINS], dt, tag="sums")
        lbT = cpool.tile([128, NTILES], dt, tag="lbT")
        rpT = cpool.tile([128, NTILES], dt, tag="rpT")
        wb = cpool.tile([1, 64], mybir.dt.int32, tag="wb")
        bias = cpool.tile([128, NBIAS], dt, tag="bias")
        iota4 = cpool.tile([128, 4], dt, tag="iota4")
        iotab = cpool.tile([128, WIN], dt, tag="iotab")
        ident = cpool.tile([128, 128], dt, tag="ident")
        wpos = cpool.tile([4, D2], dt, tag="wpos")
        w0a = [cpool.tile([128, HID], dt, tag=f"w0a{i}", name=f"w0a{i}") for i in range(NBLK)]
        w0b = [cpool.tile([128, HID], dt, tag=f"w0b{i}", name=f"w0b{i}") for i in range(NBLK)]
        w1 = [cpool.tile([128, HID], dt, tag=f"w1{i}", name=f"w1{i}") for i in range(NBLK)]
        wsa = [cpool.tile([128, HID], dt, tag=f"wsa{i}", name=f"wsa{i}") for i in range(NBLK)]
        wsb = [cpool.tile([128, HID], dt, tag=f"wsb{i}", name=f"wsb{i}") for i in range(NBLK)]
        wc = cpool.tile([128, HID], dt, tag="wc")

        nc.sync.dma_start(lbT[:], d_lbT[:])
        nc.sync.dma_start(rpT[:], d_rpT[:])
        nc.sync.dma_start(wb[:], d_wb[:])
        nc.sync.dma_start(bias[:], d_bias[:])
        nc.sync.dma_start(iota4[:], d_iota4[:])
        nc.sync.dma_start(iotab[:], d_iotab[:])
        nc.sync.dma_start(ident[:], d_ident[:])
        nc.sync.dma_start(wpos[:], d_wpos4[:])
        for i in range(NBLK):
            nc.sync.dma_start(w0a[i][:], d_w0[i, 0:128, :])
            nc.sync.dma_start(w0b[i][:], d_w0[i, 128:256, :])
            nc.sync.dma_start(w1[i][:], d_w1[i, :, :])
            nc.sync.dma_start(wsa[i][:], d_ws[i, 0:128, :])
            nc.sync.dma_start(wsb[i][:], d_ws[i, 128:256, :])
        nc.sync.dma_start(wc[:], d_wc[:])


        # ---------------- pt = 2*frac(clip(p+.5)*res) - 1, flat layout
        pflat = spool.tile([128, FW], dt, tag="oh", name="pflat")
        nc.sync.dma_start(pflat[:], d_pts[:])
        nc.vector.tensor_scalar(pflat[:], pflat[:], 0.5, 1.0 - 1e-6, OP.add, OP.min)
        nc.vector.tensor_scalar(pflat[:], pflat[:], 1e-6, float(RES), OP.max, OP.mult)
        # frac(x) = x - floor(x), robust to convert rounding mode
        ci = spool.tile([128, FW], mybir.dt.int32, tag="gbb", name="ci")
        nc.vector.tensor_copy(ci[:], pflat[:])
        cf = spool.tile([128, FW], dt, tag="rpb", name="cf")
        nc.vector.tensor_copy(cf[:], ci[:])
        nc.vector.tensor_tensor(pflat[:], pflat[:], cf[:], OP.subtract)
        m1 = spool.tile([128, FW], dt, tag="gnet", name="m1")
        nc.vector.tensor_scalar(m1[:], pflat[:], 0.0, None, OP.is_lt)
        nc.vector.tensor_tensor(pflat[:], pflat[:], m1[:], OP.add)
        nc.vector.tensor_scalar(pflat[:], pflat[:], 2.0, -1.0, OP.mult, OP.add)
        scr_flat = d_scr[:].rearrange("a (b f) -> (a b) f", f=FW)
        nc.sync.dma_start(scr_flat, pflat[:])

        def evac(dst, src, bias_col=None, gelu=False, eng="act"):
            if eng == "act":
                f = GELU if gelu else (
                    AF.Identity if bias_col is not None else AF.Copy)
                nc.scalar.activation(
                    dst, src, f,
                    bias=bias_col if bias_col is not None else 0.0)
            else:
                assert not gelu
                if bias_col is not None:
                    nc.vector.tensor_scalar(dst, src, bias_col, None, OP.add)
                else:
                    nc.vector.tensor_copy(dst, src)

        # ---------------- setup: pos-mlp + resblock 0, per 512-chunk
        for c in range(NCHUNK):
            ptc = spool.tile([4, 512], dt, tag="wstage", name="ptc")
            nc.sync.dma_start(ptc[:], d_scr[:, c * 512:(c + 1) * 512])
            x0a = psum2.tile([128, 512], dt, tag="pl")
            x0b = psum1.tile([128, 512], dt, tag="tp", bufs=2, name="x0b")
            nc.tensor.matmul(x0a[:], wpos[:, 0:128], ptc[:], start=True, stop=True)
            nc.tensor.matmul(x0b[:], wpos[:, 128:256], ptc[:], start=True, stop=True)
            gxa = spool.tile([128, 512], dt, tag="gpool", name="gxa")
            gxb = spool.tile([128, 512], dt, tag="rpool", name="gxb")
            rxa = spool.tile([128, 512], dt, tag="gnet", name="rxa")
            rxb = spool.tile([128, 512], dt, tag="gbb", name="rxb")
            evac(gxa[:], x0a[:], bias[:, 0:1], gelu=True)
            evac(gxb[:], x0b[:], bias[:, 1:2], gelu=True)
            evac(rxa[:], x0a[:], bias[:, 0:1], eng="dve")
            evac(rxb[:], x0b[:], bias[:, 1:2], eng="dve")
            hp = psum2.tile([128, 512], dt, tag="hp")
            nc.tensor.matmul(hp[:], w0a[0][:], gxa[:], start=True, stop=False)
            nc.tensor.matmul(hp[:], w0b[0][:], gxb[:], start=False, stop=True)
            gh = spool.tile([128, 512], dt, tag="gh")
            evac(gh[:], hp[:], bias[:, 2:3], gelu=True)
            npp = psum2.tile([128, 512], dt, tag="hp", name="npp")
            nc.tensor.matmul(npp[:], w1[0][:], gh[:], start=True, stop=False)
            nc.tensor.matmul(npp[:], wsa[0][:], rxa[:], start=False, stop=False)
            nc.tensor.matmul(npp[:], wsb[0][:], rxb[:], start=False, stop=True)
            evac(net[:, c * 512:(c + 1) * 512], npp[:], bias[:, 7:8], eng="dve")

        # ---------------- scatter: one-hot matmuls into dynamic bin windows
        def scatter_pass(src_of_chunk, fold_recip=False):
            nc.vector.memset(sums[:], 0.0)
            for c in range(NCHUNK):
                src = src_of_chunk(c)
                tp = psum1.tile([128, 512], dt, tag="tp", bufs=2)
                for t in range(4):
                    nc.tensor.transpose(tp[:, t * 128:(t + 1) * 128],
                                        src[:, t * 128:(t + 1) * 128], ident[:])
                ntT = spool.tile([128, 512], dt, tag="ntT")
                evac(ntT[:], tp[:], eng="act")
                oh = spool.tile([128, 4 * WIN], dt, tag="oh", name="ohs")
                for t in range(4):
                    eng = nc.vector
                    col = slice(4 * c + t, 4 * c + t + 1)
                    if fold_recip:
                        eng.tensor_scalar(oh[:, t * WIN:(t + 1) * WIN], iotab[:],
                                          lbT[:, col], rpT[:, col],
                                          OP.is_equal, OP.mult)
                    else:
                        eng.tensor_scalar(oh[:, t * WIN:(t + 1) * WIN], iotab[:],
                                          lbT[:, col], None, OP.is_equal)
                sp = psum1.tile([128, WIN], dt, tag="w512", bufs=2)
                for t in range(4):
                    nc.tensor.matmul(sp[:], ntT[:, t * 128:(t + 1) * 128],
                                     oh[:, t * WIN:(t + 1) * WIN],
                                     start=(t == 0), stop=(t == 3))
                base = load_base(c, EV.DVE)
                dst = sums[:, bass.ds(base, WIN)]
                nc.vector.tensor_tensor(dst, dst, sp[:], OP.add)

        # ---------------- pooling iterations
        for i in range(1, NBLK):
            scatter_pass(lambda c: net[:, c * 512:(c + 1) * 512])
            for c in range(NCHUNK):
                baseA = load_base(c, EV.Activation)
                wstage = spool.tile([128, WIN], dt, tag="wstage")
                nc.scalar.activation(wstage[:], sums[:, bass.ds(baseA, WIN)],
                                     AF.Copy)
                mtp = psum1.tile([128, WIN], dt, tag="w512", bufs=2, name="mtp")
                for t in range(4):
                    nc.tensor.transpose(mtp[:, t * 128:(t + 1) * 128],
                                        wstage[:, t * 128:(t + 1) * 128], ident[:])
                mT = spool.tile([128, WIN], dt, tag="ntT", name="mT")
                evac(mT[:], mtp[:], eng="dve")
                gbb = spool.tile([128, 512], dt, tag="gbb")
                nc.sync.dma_start(gbb[:], d_lbr[c:c + 1, :].to_broadcast((128, 512)))
                rpb = spool.tile([128, 512], dt, tag="rpb")
                nc.sync.dma_start(rpb[:], d_rpr[c:c + 1, :].to_broadcast((128, 512)))
                oh = spool.tile([128, 4 * 512], dt, tag="oh", name="ohg")
                for t in range(4):
                    eng = nc.vector
                    eng.scalar_tensor_tensor(oh[:, t * 512:(t + 1) * 512], gbb[:],
                                             iota4[:, t:t + 1], rpb[:],
                                             OP.is_equal, OP.mult)
                pl = psum2.tile([128, 512], dt, tag="pl")
                for t in range(4):
                    nc.tensor.matmul(pl[:], mT[:, t * 128:(t + 1) * 128],
                                     oh[:, t * 512:(t + 1) * 512],
                                     start=(t == 0), stop=(t == 3))
                gpool = spool.tile([128, 512], dt, tag="gpool")
                rpool = spool.tile([128, 512], dt, tag="rpool")
                evac(gpool[:], pl[:], gelu=True)
                evac(rpool[:], pl[:], eng="dve")
                ncur = net[:, c * 512:(c + 1) * 512]
                gnet = spool.tile([128, 512], dt, tag="gnet")
                evac(gnet[:], ncur, gelu=True)
                hp = psum2.tile([128, 512], dt, tag="hp")
                nc.tensor.matmul(hp[:], w0a[i][:], gnet[:], start=True, stop=False)
                nc.tensor.matmul(hp[:], w0b[i][:], gpool[:], start=False, stop=True)
                gh = spool.tile([128, 512], dt, tag="gh")
                evac(gh[:], hp[:], bias[:, 2 + i:3 + i], gelu=True)
                npp = psum2.tile([128, 512], dt, tag="hp", name="npp")
                nc.tensor.matmul(npp[:], w1[i][:], gh[:], start=True, stop=False)
                nc.tensor.matmul(npp[:], wsa[i][:], ncur, start=False, stop=False)
                nc.tensor.matmul(npp[:], wsb[i][:], rpool[:], start=False, stop=True)
                evac(ncur, npp[:], bias[:, 7 + i:8 + i], eng="dve")

        # ---------------- head: c = net @ Wc + b_c, scatter, normalize, out
        def head_chunk(c):
            cp = psum2.tile([128, 512], dt, tag="hp")
            nc.tensor.matmul(cp[:], wc[:], net[:, c * 512:(c + 1) * 512],
                             start=True, stop=True)
            cv = spool.tile([128, 512], dt, tag="wstage", name="cv")
            evac(cv[:], cp[:], bias[:, 12:13], eng="act")
            return cv[:]

        scatter_pass(head_chunk, fold_recip=True)
        nc.sync.dma_start(d_out[:], sums[:])

    return nc


# ================================================================ run + glue
_BUILT = {}


def get_nc():
    if "nc" not in _BUILT:
        nc = build_bass()
        nc.compile()          # bacc pipeline: reg alloc, library loads, ...
        _BUILT["nc"] = nc
    return _BUILT["nc"]


def make_in_maps(p, sparse_coords, W_pos, b_pos, W0, b0, W1, b1, Ws, Wc, b_c, res):
    index, counts = point_meta(p, sparse_coords, int(res))
    shards = shard(np.asarray(p, F32), index)
    wdict = weight_inputs(W_pos, b_pos, W0, b0, W1, b1, Ws, Wc, b_c)
    in_maps = []
    for sh in shards:
        ci = core_inputs(sh)
        m = dict(pts_flat=ci["pts_flat"], lbT=ci["lbT"], lb_rows=ci["lb_rows"],
                 wbase=ci["wbase"], rpT=ci["rpT"], rp_rows=ci["rp_rows"],
                 wpos4=wdict["wpos4"], w0=wdict["w0"], w1=wdict["w1"],
                 ws=wdict["ws"], wc=wdict["wc"], bias=wdict["bias"],
                 iota4=wdict["iota4"], iota_bc=wdict["iota_bc"],
                 ident=wdict["ident"])
        in_maps.append(m)
    return in_maps, shards, counts


def assemble(results, shards, counts, sparse_coords):
    sc = np.asarray(sparse_coords)
    starts = np.concatenate([[0], np.cumsum(counts)[:-1]])
    out = np.zeros((sc.shape[0], HID), F32)
    for sh, r_ in zip(shards, results):
        tab = np.asarray(r_["out_grid"])              # [128, NBINS]
        lo, hi, b = sh["lo"], sh["hi"], sh["batch"]
        hi_eff = min(hi, int(counts[b]))
        if hi_eff > lo:
            out[starts[b] + lo: starts[b] + hi_eff] = tab[:, 0:hi_eff - lo].T
    return out


def kernel(p, sparse_coords, W_pos, b_pos, W0, b0, W1, b1, Ws, Wc, b_c, res):
    if "/opt/trn_rl_repo" not in sys.path:
        sys.path.insert(0, "/opt/trn_rl_repo")
    from concourse.bass_utils import run_bass_kernel_spmd

    in_maps, shards, counts = make_in_maps(
        p, sparse_coords, W_pos, b_pos, W0, b0, W1, b1, Ws, Wc, b_c, res)
    nc = get_nc()
    results = run_bass_kernel_spmd(nc, in_maps, list(range(NCORES))).results
    return assemble(results, shards, counts, sparse_coords)

